# revision 1
# baseline (speedup 1.0000x reference)
"""Trainium2 Bass kernel for the scalar-input GRU (B=512, T=128, H=512) + ReLU/Linear head.

Data-parallel over batch across 8 NeuronCores (64 rows each); per core the 64
rows run as W=2 interleaved waves of 32 so one wave's gate algebra overlaps the
other wave's matmuls.

Transposed compute layout (weights stationary): per-step tensors live as
[j (gate-unit) partitions, batch free].  gh = W h uses lhsT = W-chunk
[128(k), 128(j)] stationary, rhs = hT chunk [128(k), BW(b)] moving ->
out [128(j), BW] PSUM, accumulated over k.  The TimelineSim cost model charges
matmuls by output free size only, so this more than halves PE time vs
streaming the weights, and h' is produced directly in the layout the next
step's matmuls consume (no PE transposes in the loop).

Per wave-step: psR/psZ/psNG live in separate PSUM banks (cross-engine deps are
tile-granular, and at most ONE accumulation group may be open per bank at a
time — a second start in the same bank silently drops the open group's partial
sum).  Groups are opened/closed strictly sequentially per bank.

Gate algebra: sigmoid(r) -> m = r*pre_n (bf16) -> G pairs on PE (gx_n aug +
identity-matmul accumulate of m) -> tanh -> u = c*n, h' = u + z*h, with
sigmoid(z)/c = 1-z/w = z*h off the critical path.  The bf16 tail runs on DVE
in 2x perf mode; biases/x/weights are plain bf16 (tolerance 2e-2).
"""

import sys

sys.path.insert(0, "/opt/trn_rl_repo")

import numpy as np

import concourse.bacc as bacc
import concourse.bass as bass
import concourse.mybir as mybir
import concourse.tile as tile
from concourse.bass_utils import run_bass_kernel_spmd
from concourse.masks import make_identity

N_CORES = 8
B_FULL, T_FULL, H = 512, 128, 512
B = B_FULL // N_CORES  # 64 batch rows per core
W = 2  # waves per core
BW = B // W  # 32 rows per wave
G3 = 3 * H  # 1536
NK = H // 128  # 4 contraction chunks
NJ = 4  # j-chunks per gate (H/128)
F32 = mybir.dt.float32
BF16 = mybir.dt.bfloat16
AF = mybir.ActivationFunctionType


def build_nc(T: int = T_FULL) -> bass.Bass:
    nc = bacc.Bacc("TRN2", target_bir_lowering=False, debug=False)

    x_d = nc.dram_tensor("x", [B, T], F32, kind="ExternalInput")
    whh_d = nc.dram_tensor("w_hh", [G3, H], F32, kind="ExternalInput")
    wih_d = nc.dram_tensor("w_ih", [G3, 1], F32, kind="ExternalInput")
    bih_d = nc.dram_tensor("b_ih", [G3], F32, kind="ExternalInput")
    bhh_d = nc.dram_tensor("b_hh", [G3], F32, kind="ExternalInput")
    fcw_d = nc.dram_tensor("fc_w", [1, H], F32, kind="ExternalInput")
    fcb_d = nc.dram_tensor("fc_b", [1], F32, kind="ExternalInput")
    out_d = nc.dram_tensor("out", [B, 1], F32, kind="ExternalOutput")

    with tile.TileContext(nc) as tc:
        _body(tc, T, x_d, whh_d, wih_d, bih_d, bhh_d, fcw_d, fcb_d, out_d)
    nc.compile()
    return nc


def _body(tc, T, x_d, whh_d, wih_d, bih_d, bhh_d, fcw_d, fcb_d, out_d):
    nc = tc.nc
    with (
        tc.tile_pool(name="const", bufs=1) as cpool,
        tc.tile_pool(name="state", bufs=3) as spool,
        tc.tile_pool(name="work", bufs=3) as wpool,
        tc.tile_pool(name="psmain", bufs=2, space="PSUM") as ppool,
    ):
        # ---- one-time prep ----
        # w_hh staged first (it gates the transposes), in 4 chunk DMAs so the
        # first transposes start while the rest streams in
        wstage = cpool.tile([128, (G3 // 128) * H], F32)
        for cg in range(12):
            nc.sync.dma_start(
                out=wstage[:, cg * H : (cg + 1) * H],
                in_=whh_d[cg * 128 : (cg + 1) * 128, :],
            )

        ident128 = cpool.tile([128, 128], F32)
        make_identity(nc, ident128)
        identb = cpool.tile([128, 128], BF16)
        nc.vector.tensor_copy(identb[:, :], ident128[:, :])
        ident64 = cpool.tile([64, 64], F32)
        make_identity(nc, ident64)

        # small input loads, spread over issue queues
        x_sb = cpool.tile([B, T], F32)
        nc.scalar.dma_start(out=x_sb[:, :], in_=x_d[:, :])
        wi12 = cpool.tile([12, 128], F32)
        nc.scalar.dma_start(
            out=wi12[:, :], in_=wih_d[:, :].rearrange("(p c) one -> p (c one)", p=12)
        )
        bs12 = cpool.tile([12, 128], F32)
        nc.gpsimd.dma_start(
            out=bs12[:, :], in_=bhh_d[None, :].rearrange("one (p c) -> (one p) c", p=12)
        )
        bi12 = cpool.tile([12, 128], F32)
        nc.gpsimd.dma_start(
            out=bi12[:, :], in_=bih_d[None, :].rearrange("one (p c) -> (one p) c", p=12)
        )
        fcwf = cpool.tile([128, NK], F32)
        nc.scalar.dma_start(
            out=fcwf[:, :],
            in_=fcw_d[:, :]
            .rearrange("one (k p) -> one k p", p=128)
            .transpose([2, 0, 1])
            .rearrange("p one k -> p (one k)"),
        )
        fcbf = cpool.tile([1, 1], F32)
        nc.gpsimd.dma_start(out=fcbf[:, :], in_=fcb_d[None, :])
        onesf = cpool.tile([1, B], F32)
        nc.gpsimd.memset(onesf[:, :], 1.0)

        # bsum = b_hh + b_ih on the r/z rows
        nc.vector.tensor_add(bs12[0:8, :], bs12[0:8, :], bi12[0:8, :])

        # bf16 casts of (wi, bs, bih) into one shared tile, one DMA to DRAM
        hi_all = cpool.tile([12, 384], BF16)
        nc.gpsimd.tensor_copy(hi_all[:, 0:128], wi12[:, :])
        nc.gpsimd.tensor_copy(hi_all[:, 128:256], bs12[:, :])
        nc.gpsimd.tensor_copy(hi_all[:, 256:384], bi12[:, :])
        scr_d = nc.dram_tensor("scr_aug", [3, 12, 128], BF16, kind="Internal")
        nc.sync.dma_start(
            out=scr_d[:, :, :].transpose([1, 0, 2]),
            in_=hi_all[:, :].rearrange("p (k c) -> p k c", k=3),
        )

        # xaug rows: (x, 1); ones via packed-uint32 memset on the idle Pool
        # engine, x row DMA-overwritten below.  Tolerance (2e-2) allows plain
        # bf16 x/weights/biases — no hi/lo pair splits.
        xaug = cpool.tile([2, T * B], BF16)
        nc.gpsimd.memset(xaug[:, :].bitcast(mybir.dt.uint32), 0x3F803F80)
        ones1 = cpool.tile([1, B], BF16)
        nc.gpsimd.memset(ones1[:, :], 1.0)

        # x PE-transposed into (t, b) order, staged via DRAM
        xt_ps = ppool.tile([T, B], F32, tag="psPREP", bufs=2, name="xt_ps")
        nc.tensor.transpose(xt_ps[:, :], x_sb[:, :], ident64)
        xt_b = cpool.tile([T, B], BF16)
        nc.vector.tensor_copy(xt_b[:, :], xt_ps[:, :])
        xt_scr = nc.dram_tensor("xt_scr", [T, B], BF16, kind="Internal")
        nc.scalar.dma_start(out=xt_scr[:, :], in_=xt_b[:, :])
        nc.sync.dma_start(
            out=xaug[0:1, :], in_=xt_scr[:, :].rearrange("p c -> (p c)")[None, :]
        )

        # Stationary aug tiles (PE needs partition base 0/32/64):
        #   AUG [2, G3]: (wi, bsum), rhs = xaug (x, 1)
        #   AUGNB [1, H]: (bsum_n), rhs = ones1
        #   AUGG [2, H]: (wi_n, bih_n), rhs = xaug
        AUG = cpool.tile([2, G3], BF16)
        AUGNB = cpool.tile([1, H], BF16)
        AUGG = cpool.tile([2, H], BF16)

        def row_dma(q, dst, r, kind, p0, p1):
            q.dma_start(
                out=dst[r : r + 1, :],
                in_=scr_d[kind, p0:p1, :].rearrange("p c -> (p c)")[None, :],
            )

        # kinds: 0 = wi, 1 = bs, 2 = bih
        row_dma(nc.sync, AUG, 0, 0, 0, 12)
        row_dma(nc.sync, AUG, 1, 1, 0, 12)
        row_dma(nc.scalar, AUGNB, 0, 1, 8, 12)
        row_dma(nc.scalar, AUGG, 0, 0, 8, 12)
        row_dma(nc.scalar, AUGG, 1, 2, 8, 12)

        # w_hh.T chunks: wT[p, k*G3 + j] = w_hh[j, 128k + p]  (bf16)
        # transposes rotate over 6 idle recurrence banks; evacuation copies
        # round-robin over DVE/ACT
        wT = cpool.tile([128, NK * G3], BF16)
        prep_tags = ["psR0", "psZ0", "psNG0", "psR1", "psZ1", "psNG1"]
        copy_engines = [nc.vector.tensor_copy,
                        lambda o, i: nc.scalar.activation(o, i, AF.Copy)]
        pi = 0
        for c in range(G3 // 128):
            for k in range(NK):
                tp = ppool.tile([128, 128], F32, tag=prep_tags[pi % 6], bufs=1,
                                name=f"wprep_{c}_{k}")
                nc.tensor.transpose(
                    tp[:, :], wstage[:, c * H + k * 128 : c * H + (k + 1) * 128], ident128
                )
                copy_engines[pi % 2](
                    wT[:, k * G3 + c * 128 : k * G3 + (c + 1) * 128], tp[:, :]
                )
                pi += 1

        # state init: hT[w] [128, NK*BW] bf16, col block k = h dims [128k:128k+128)
        hT = []
        for w in range(W):
            h0 = spool.tile([128, NK * BW], BF16, tag=f"hT{w}", name=f"hT{w}_init")
            nc.gpsimd.memset(h0[:, :], 0.0)
            hT.append(h0)

        def emit_mms(w, t, ps):
            # One open accumulation group per PSUM bank at a time; groups
            # sequential per bank.  Gate order r, z, n.
            psr, psz, psng = ps
            prz = (psr, psz)
            xs = xaug[0:2, t * B + w * BW : t * B + (w + 1) * BW]
            for g in (0, 1):
                for jc in range(NJ):
                    nc.tensor.matmul(
                        prz[g][:, jc * BW : (jc + 1) * BW],
                        AUG[0:2, g * H + jc * 128 : g * H + (jc + 1) * 128],
                        xs,
                        start=True,
                        stop=(t == 0),
                    )
                    if t > 0:
                        for k in range(NK):
                            nc.tensor.matmul(
                                prz[g][:, jc * BW : (jc + 1) * BW],
                                wT[:, k * G3 + g * H + jc * 128 : k * G3 + g * H + (jc + 1) * 128],
                                hT[w][:, k * BW : (k + 1) * BW],
                                start=False,
                                stop=(k == NK - 1),
                            )
            for jc in range(NJ):
                nc.tensor.matmul(
                    psng[:, jc * BW : (jc + 1) * BW],
                    AUGNB[0:1, jc * 128 : (jc + 1) * 128],
                    ones1[0:1, w * BW : (w + 1) * BW],
                    start=True,
                    stop=(t == 0),
                )
                if t > 0:
                    for k in range(NK):
                        nc.tensor.matmul(
                            psng[:, jc * BW : (jc + 1) * BW],
                            wT[:, k * G3 + 2 * H + jc * 128 : k * G3 + 2 * H + (jc + 1) * 128],
                            hT[w][:, k * BW : (k + 1) * BW],
                            start=False,
                            stop=(k == NK - 1),
                        )

        def emit_algebra1(w, t, ps, st):
            psr, psz, psng = ps
            # sigmoid(r) alone on the critical path; sigmoid(z) right after
            # (fills ACT idle time); c/w in bf16 2x mode on DVE, off-path
            rz = wpool.tile([128, 8 * BW], BF16, tag=f"rz{w}", name=f"rz{w}_{t}")
            nc.scalar.activation(rz[:, 0 : 4 * BW], psr[:, :], AF.Sigmoid)
            m = wpool.tile([128, 4 * BW], BF16, tag=f"m{w}", name=f"m{w}_{t}")
            nc.vector.tensor_mul(m[:, :], rz[:, 0 : 4 * BW], psng[:, 0 : 4 * BW])
            nc.scalar.activation(rz[:, 4 * BW : 8 * BW], psz[:, :], AF.Sigmoid)
            c = wpool.tile([128, 4 * BW], BF16, tag=f"c{w}", name=f"c{w}_{t}")
            nc.vector.tensor_scalar(
                c[:, :], rz[:, 4 * BW : 8 * BW], 1.0, -1.0,
                mybir.AluOpType.subtract, mybir.AluOpType.mult,
            )
            wv = wpool.tile([128, 4 * BW], BF16, tag=f"w{w}", name=f"w{w}_{t}")
            nc.vector.tensor_mul(wv[:, :], rz[:, 4 * BW : 8 * BW], hT[w][:, :])
            st["rz"], st["c"], st["wv"], st["m"] = rz, c, wv, m

        def emit_gpairs(w, t, ps, st):
            # G groups as adjacent aug/ident pairs on PE: ps_g = gx_n, += m
            psr, psz, psng = ps
            m = st["m"]
            xs = xaug[0:2, t * B + w * BW : t * B + (w + 1) * BW]
            for jc in range(NJ):
                nc.tensor.matmul(
                    psng[:, (4 + jc) * BW : (5 + jc) * BW],
                    AUGG[0:2, jc * 128 : (jc + 1) * 128],
                    xs,
                    start=True,
                    stop=False,
                )
                nc.tensor.matmul(
                    psng[:, (4 + jc) * BW : (5 + jc) * BW],
                    identb[:, :],
                    m[:, jc * BW : (jc + 1) * BW],
                    start=False,
                    stop=True,
                )

        def emit_algebra2(w, t, ps, st):
            psr, psz, psng = ps
            rz, c, wv = st["rz"], st["c"], st["wv"]
            n = wpool.tile([128, 4 * BW], BF16, tag=f"n{w}", name=f"n{w}_{t}")
            nc.scalar.activation(n[:, :], psng[:, 4 * BW : 8 * BW], AF.Tanh)
            u = wpool.tile([128, 4 * BW], BF16, tag=f"u{w}", name=f"u{w}_{t}")
            nc.vector.tensor_mul(u[:, :], c[:, :], n[:, :])
            hn = spool.tile([128, NK * BW], BF16, tag=f"hT{w}", name=f"hT{w}_{t}")
            nc.vector.tensor_add(hn[:, :], u[:, :], wv[:, :])
            hT[w] = hn

        # ---- the recurrence, fully unrolled, 2 waves interleaved ----
        sts = [{}, {}]
        pss = [None, None]
        prev_pss = [None, None]
        for t in range(T):
            for w in range(W):
                prev_pss[w] = pss[w]
                psr = ppool.tile(
                    [128, 4 * BW], F32, tag=f"psR{w}", bufs=1, name=f"psr{w}_{t}"
                )
                psz = ppool.tile(
                    [128, 4 * BW], F32, tag=f"psZ{w}", bufs=1, name=f"psz{w}_{t}"
                )
                psng = ppool.tile(
                    [128, 8 * BW], F32, tag=f"psNG{w}", bufs=1, name=f"psng{w}_{t}"
                )
                pss[w] = (psr, psz, psng)
            if t > 0:
                emit_gpairs(1, t - 1, prev_pss[1], sts[1])
                emit_algebra2(1, t - 1, prev_pss[1], sts[1])
            emit_mms(0, t, pss[0])
            emit_algebra1(0, t, pss[0], sts[0])
            emit_mms(1, t, pss[1])
            emit_gpairs(0, t, pss[0], sts[0])
            emit_algebra2(0, t, pss[0], sts[0])
            emit_algebra1(1, t, pss[1], sts[1])
        emit_gpairs(1, T - 1, pss[1], sts[1])
        emit_algebra2(1, T - 1, pss[1], sts[1])

        # ---- head: out = relu(h) @ fc_w.T + fc_b ----
        # both waves' fc matmuls target one PSUM tile at partition bases 0/32
        # so the epilogue is a single copy + a single output DMA
        pso = ppool.tile([B, 1], F32, tag="psPREP", bufs=2, name="ps_fc")
        for w in range(W):
            reluh = wpool.tile([128, NK * BW], F32, tag=f"relu{w}", name=f"relu{w}")
            nc.scalar.activation(reluh[:, :], hT[w][:, :], AF.Relu)
            po = pso[w * BW : (w + 1) * BW, :]
            nc.tensor.matmul(
                po, onesf[:, 0:BW], fcbf[0:1, 0:1], start=True, stop=False
            )
            for k in range(NK):
                nc.tensor.matmul(
                    po,
                    reluh[:, k * BW : (k + 1) * BW],
                    fcwf[:, k : k + 1],
                    start=False,
                    stop=(k == NK - 1),
                )
        outw = wpool.tile([B, 1], F32, tag="outw", name="out_sb")
        nc.vector.tensor_copy(outw[:, :], pso[:, :])
        nc.sync.dma_start(out=out_d[:, :], in_=outw[:, :])


_NC_CACHE: dict[int, bass.Bass] = {}


def _get_nc(T: int = T_FULL) -> bass.Bass:
    if T not in _NC_CACHE:
        _NC_CACHE[T] = build_nc(T)
    return _NC_CACHE[T]


def kernel(x, w_ih, w_hh, b_ih, b_hh, fc_w, fc_b, _trace=False, _tmpdir=None):
    x = np.ascontiguousarray(np.asarray(x, dtype=np.float32))
    nc = _get_nc(x.shape[1])
    shared = {
        "w_hh": np.ascontiguousarray(np.asarray(w_hh, np.float32)),
        "w_ih": np.ascontiguousarray(np.asarray(w_ih, np.float32)),
        "b_ih": np.ascontiguousarray(np.asarray(b_ih, np.float32)),
        "b_hh": np.ascontiguousarray(np.asarray(b_hh, np.float32)),
        "fc_w": np.ascontiguousarray(np.asarray(fc_w, np.float32)),
        "fc_b": np.ascontiguousarray(np.asarray(fc_b, np.float32)),
    }
    in_maps = [{"x": x[c * B : (c + 1) * B], **shared} for c in range(N_CORES)]
    res = run_bass_kernel_spmd(
        nc, in_maps, list(range(N_CORES)), trace=_trace, tmpdir=_tmpdir
    )
    out = np.concatenate([res.results[c]["out"] for c in range(N_CORES)], axis=0)
    if _trace:
        return out, res
    return out

